# revision 2
# baseline (speedup 1.0000x reference)
"""CenterLoss on 8 Trainium2 NeuronCores.

mean_i ||x_i - centers[labels_i]||^2  with per-sample clip to [1e-12, 1e12].

Sharding (expert/tensor-style, per the class-sharding strategy):
  - centers is sharded over classes: core j owns rows [j*12500, (j+1)*12500),
    plus one appended zero row used as the gather target for padding.
  - the batch is routed to the core that owns each sample's label class
    (MoE-style dispatch done while building the per-core input shards).
  - each core gathers its centers rows ON DEVICE via indirect DMA
    (128 row-offsets per descriptor-generated transfer), computes
    per-sample squared distances, clips, and reduces to a partial sum.
  - the 8 partial sums are combined on the host (gather/unshard step).

Per-core device kernel (SPMD, identical program, T row-tiles of 128):
  xa   [128, T*512] f32 : x rows, tile t in columns [t*512,(t+1)*512)
  idx  [128, T]     i32 : shard-local center row per sample (12500 = zero row)
  ctab [12501, 512] f32 : this core's class shard + zero row
  out  [1, 1]       f32 : sum_t clip(||x - c||^2)
"""

import os
import sys

import numpy as np

if "/opt/trn_rl_repo" not in sys.path:
    sys.path.insert(0, "/opt/trn_rl_repo")

N_CORES = 8
C = 100000
D = 512
P = 128
CPC = C // N_CORES  # classes per core
T_MIN = 5  # covers per-core counts up to 640; Binomial(4096,1/8) never exceeds this

_compiled = {}
last_results = None  # BassKernelResults of the most recent run (for test harnesses)


def _build(T):
    import concourse.bass as bass
    import concourse.tile as tile
    from concourse import bacc, mybir

    nc = bacc.Bacc("TRN2", target_bir_lowering=False, debug=False, num_devices=N_CORES)
    xa_d = nc.dram_tensor("xa", [P, T * D], mybir.dt.float32, kind="ExternalInput").ap()
    idx_d = nc.dram_tensor("idx", [P, T], mybir.dt.int32, kind="ExternalInput").ap()
    ctab_d = nc.dram_tensor(
        "ctab", [CPC + 1, D], mybir.dt.float32, kind="ExternalInput"
    ).ap()
    out_d = nc.dram_tensor("out", [1, 1], mybir.dt.float32, kind="ExternalOutput").ap()

    with tile.TileContext(nc) as tc:
        with (
            tc.tile_pool(name="work", bufs=3) as work,
            tc.tile_pool(name="small", bufs=1) as small,
            tc.tile_pool(name="psum", bufs=1, space="PSUM") as psum_pool,
        ):
            idx_t = small.tile([P, T], mybir.dt.int32)
            nc.sync.dma_start(idx_t[:], idx_d[:])
            dist = small.tile([P, T], mybir.dt.float32)

            for t in range(T):
                x_t = work.tile([P, D], mybir.dt.float32, tag="x")
                nc.sync.dma_start(x_t[:], xa_d[:, t * D : (t + 1) * D])
                c_t = work.tile([P, D], mybir.dt.float32, tag="c")
                nc.gpsimd.indirect_dma_start(
                    out=c_t[:],
                    out_offset=None,
                    in_=ctab_d[:],
                    in_offset=bass.IndirectOffsetOnAxis(ap=idx_t[:, t : t + 1], axis=0),
                )
                diff = work.tile([P, D], mybir.dt.float32, tag="diff")
                nc.vector.tensor_tensor(
                    out=diff[:], in0=x_t[:], in1=c_t[:], op=mybir.AluOpType.subtract
                )
                sq = work.tile([P, D], mybir.dt.float32, tag="sq")
                nc.scalar.activation(
                    out=sq[:],
                    in_=diff[:],
                    func=mybir.ActivationFunctionType.Square,
                    accum_out=dist[:, t : t + 1],
                )

            distc = small.tile([P, T], mybir.dt.float32)
            nc.vector.tensor_scalar(
                out=distc[:],
                in0=dist[:],
                scalar1=1e-12,
                scalar2=1e12,
                op0=mybir.AluOpType.max,
                op1=mybir.AluOpType.min,
            )
            s = small.tile([P, 1], mybir.dt.float32)
            nc.vector.reduce_sum(out=s[:], in_=distc[:], axis=mybir.AxisListType.X)
            ones = small.tile([P, 1], mybir.dt.float32)
            nc.vector.memset(ones[:], 1.0)
            ps = psum_pool.tile([1, 1], mybir.dt.float32)
            nc.tensor.matmul(ps[:], lhsT=s[:], rhs=ones[:], start=True, stop=True)
            res = small.tile([1, 1], mybir.dt.float32)
            nc.vector.tensor_copy(res[:], ps[:])
            nc.sync.dma_start(out_d[:], res[:])

    nc.compile()
    return nc


def _get_compiled(T):
    if T not in _compiled:
        _compiled[T] = _build(T)
    return _compiled[T]


def make_in_maps(x, labels, centers):
    """Shard full inputs into per-core input maps. Returns (in_maps, T, B)."""
    x = np.asarray(x, dtype=np.float32)
    labels = np.asarray(labels).astype(np.int64)
    centers = np.asarray(centers, dtype=np.float32)
    B = x.shape[0]

    owner = labels // CPC
    counts = np.bincount(owner, minlength=N_CORES)
    T = max(T_MIN, -(-int(counts.max()) // P))
    n_pad = T * P

    zero_row = np.zeros((1, D), np.float32)
    in_maps = []
    for j in range(N_CORES):
        sel = np.nonzero(owner == j)[0]
        k = len(sel)
        xj = np.zeros((n_pad, D), np.float32)
        xj[:k] = x[sel]
        ij = np.full((n_pad,), CPC, np.int32)
        ij[:k] = (labels[sel] - j * CPC).astype(np.int32)
        xa = np.ascontiguousarray(
            xj.reshape(T, P, D).transpose(1, 0, 2).reshape(P, T * D)
        )
        idx = np.ascontiguousarray(ij.reshape(T, P).T)
        ctab = np.concatenate([centers[j * CPC : (j + 1) * CPC], zero_row], axis=0)
        in_maps.append({"xa": xa, "idx": idx, "ctab": ctab})
    return in_maps, T, B


def kernel(x, labels, centers):
    global last_results
    from concourse.bass_utils import run_bass_kernel_spmd

    in_maps, T, B = make_in_maps(x, labels, centers)
    nc = _get_compiled(T)

    trace = bool(os.environ.get("CENTERLOSS_TRACE"))
    kwargs = {}
    if trace:
        kwargs["tmpdir"] = os.environ.get("CENTERLOSS_TRACE_DIR") or None
    res = run_bass_kernel_spmd(
        nc, in_maps, list(range(N_CORES)), trace=trace, **kwargs
    )
    last_results = res
    total = sum(float(res.results[j]["out"][0, 0]) for j in range(N_CORES))
    return np.float32(total / B)
